# revision 27
# baseline (speedup 1.0000x reference)
"""Trainium2 Bass kernel for nn_FGCF (attention + GCN + all-pairs MLP scorer).

Self-contained: takes FULL inputs, shards across 8 NeuronCores internally,
returns FULL outputs (score, eu, el).

Design (2 NEFF launches, no collectives):
  Phase A (users sharded by core; item path replicated):
    - attention MLP z = tanh(se@W1+b1)@W2 over each core's 65536 gathered
      rows (host-gathered + transposed + bf16, as the user-axis shard),
      softmax-denominator per user via ACT exp+accum -> beta -> s2d slice.
    - item GCN (dense normalized adjacency, transposed layout) -> elT,
      pre_vT = lin1b.T @ elT + b1.
  Host: concat s2d slices, permute user axis per core.
  Phase B (users sharded; GCN replicated):
    - user GCN -> euT, pre_uT
    - all-pairs scorer: h1T = relu(pre_vT + pre_uT[:,u]) (ACT/DVE split),
      h2T = lin2.T @ h1T (PE, f32r, 2 users packed via col-groups),
      score = relu(outW.T @ relu(h2T) + out_b) (PE block-diag + fused bias).
"""
import numpy as np
import ml_dtypes

import concourse.bacc as bacc
import concourse.tile as tile
import concourse.mybir as mybir
from concourse.bass_utils import run_bass_kernel_spmd

F32 = mybir.dt.float32
F32R = mybir.dt.float32r
BF16 = mybir.dt.bfloat16
AF = mybir.ActivationFunctionType
OP = mybir.AluOpType

# precision/speed tiers: scorer_bf16 trades ~3e-3 score rel-err for ~25us;
# gcn_bf16 trades ~1e-3 eu/el rel-err for ~15us.
MODE = {"scorer_bf16": False, "gcn_bf16": False}

U, I, D = 512, 1024, 64
NCORES = 8
UB = U // NCORES          # 64 users per core
POS = UB * I              # 65536 positions per core
NT = UB                   # 64 tiles (1 user each): (128, 512) two-chunk stack
NG = NT // 4              # 16 groups of 4 tiles


def _f32(x):
    return np.ascontiguousarray(np.asarray(x, dtype=np.float32))


def _bf16(x):
    return np.ascontiguousarray(np.asarray(x).astype(ml_dtypes.bfloat16))


# ---------------------------------------------------------------- phase A ---

def build_phase_a():
    nc = bacc.Bacc("TRN2", target_bir_lowering=False, debug=False,
                   num_devices=NCORES)
    dt = nc.dram_tensor
    xT = dt("xT", [128, POS // 2], BF16, kind="ExternalInput")
    w1blk = dt("w1blk", [4, 128, 128], BF16, kind="ExternalInput")
    w2sel = dt("w2sel", [128, 8], BF16, kind="ExternalInput")
    b1blk = dt("b1blk", [128, 1], F32, kind="ExternalInput")
    se_last = dt("se_last", [UB, D], F32, kind="ExternalInput")
    ue = dt("ue", [UB, D], F32, kind="ExternalInput")
    peT = dt("peT", [D, I], F32, kind="ExternalInput")
    gdt = BF16 if MODE["gcn_bf16"] else F32R
    avT = dt("avT", [I, I], gdt, kind="ExternalInput")
    wi1 = dt("wi1", [64, 64], F32, kind="ExternalInput")
    wi2 = dt("wi2", [64, 32], F32, kind="ExternalInput")
    wi3 = dt("wi3", [32, 16], F32, kind="ExternalInput")
    bi1 = dt("bi1", [64, 1], F32, kind="ExternalInput")
    bi2 = dt("bi2", [32, 1], F32, kind="ExternalInput")
    bi3 = dt("bi3", [16, 1], F32, kind="ExternalInput")
    l1b = [dt(f"l1b{i}", [k, 128], F32, kind="ExternalInput")
           for i, k in enumerate([64, 64, 32, 64, 16, 32])]
    l1bias = dt("l1bias", [128, 1], F32, kind="ExternalInput")

    s2d_out = dt("s2d_out", [UB, D], F32, kind="ExternalOutput")
    elT_out = dt("elT_out", [272, I], F32, kind="ExternalOutput")
    preVT_out = dt("preVT_out", [128, I], F32, kind="ExternalOutput")

    with tile.TileContext(nc) as tc:
        with tc.tile_pool(name="cpool", bufs=1) as cp, \
             tc.tile_pool(name="xpool", bufs=3) as xp, \
             tc.tile_pool(name="wpool", bufs=2) as wp, \
             tc.tile_pool(name="gsb", bufs=1) as gsb, \
             tc.tile_pool(name="apsum", bufs=2, space="PSUM") as aps, \
             tc.tile_pool(name="zpsum", bufs=2, space="PSUM") as zps, \
             tc.tile_pool(name="gpsum", bufs=1, space="PSUM") as gps, \
             tc.tile_pool(name="tpsum", bufs=2, space="PSUM") as tps, \
             tc.tile_pool(name="dscr", bufs=1, space="DRAM") as dsc:

            # ---- constants ----
            w1q = []
            for j in range(4):
                t = cp.tile([128, 128], BF16, name=f"w1q{j}")
                nc.sync.dma_start(t[:], w1blk.ap()[j])
                w1q.append(t)
            w2 = cp.tile([128, 8], BF16, name="w2")
            nc.sync.dma_start(w2[:], w2sel.ap())
            b1t = cp.tile([128, 1], F32, name="b1t")
            nc.sync.dma_start(b1t[:], b1blk.ap())
            sel_t = cp.tile([UB, D], F32, name="sel_t")
            nc.sync.dma_start(sel_t[:], se_last.ap())
            ue_t = cp.tile([UB, D], F32, name="ue_t")
            nc.sync.dma_start(ue_t[:], ue.ap())
            peT_r = cp.tile([D, I], F32R, name="peT_r")
            nc.sync.dma_start(peT_r[:], peT.ap().bitcast(F32R))
            wi1_r = cp.tile([64, 64], F32R, name="wi1_r")
            nc.sync.dma_start(wi1_r[:], wi1.ap().bitcast(F32R))
            wi2_r = cp.tile([64, 32], F32R, name="wi2_r")
            nc.sync.dma_start(wi2_r[:], wi2.ap().bitcast(F32R))
            wi3_r = cp.tile([32, 16], F32R, name="wi3_r")
            nc.sync.dma_start(wi3_r[:], wi3.ap().bitcast(F32R))
            bi1_t = cp.tile([64, 1], F32, name="bi1_t")
            nc.sync.dma_start(bi1_t[:], bi1.ap())
            bi2_t = cp.tile([32, 1], F32, name="bi2_t")
            nc.sync.dma_start(bi2_t[:], bi2.ap())
            bi3_t = cp.tile([16, 1], F32, name="bi3_t")
            nc.sync.dma_start(bi3_t[:], bi3.ap())
            l1b_r = []
            for i, k in enumerate([64, 64, 32, 64, 16, 32]):
                t = cp.tile([k, 128], F32R, name=f"l1b_r{i}")
                nc.sync.dma_start(t[:], l1b[i].ap().bitcast(F32R))
                l1b_r.append(t)
            l1bias_t = cp.tile([128, 1], F32, name="l1bias_t")
            nc.sync.dma_start(l1bias_t[:], l1bias.ap())
            av_r = []
            for k in range(8):
                t = cp.tile([128, I], gdt, name=f"av_r{k}")
                nc.sync.dma_start(t[:], avT.ap()[128 * k:128 * (k + 1), :])
                av_r.append(t)

            # ---- attention: z + softmax denominators ----
            # software-pipelined: W2-combine/exp for group g-1 are emitted
            # between group g's matmuls so the PE stream never stalls.
            Dacc = gsb.tile([8, NG], F32, name="Dacc")
            Zlast = gsb.tile([8, NG], F32, name="Zlast")
            pend = []

            def z_tail(g, T):
                zb = zps.tile([8, 512], F32, name="zb")
                nc.tensor.matmul(zb[:], w2[:], T[:], start=True, stop=True)
                E = wp.tile([8, 512], F32, name="E")
                nc.scalar.activation(E[:], zb[:], AF.Exp,
                                     accum_out=Dacc[:, g:g + 1])
                nc.vector.tensor_copy(Zlast[:, g:g + 1], E[:, 511:512])

            for g in range(NG):
                xg = xp.tile([128, 2048], BF16, name="xg")
                nc.sync.dma_start(xg[:], xT.ap()[:, 2048 * g:2048 * (g + 1)])
                bank = aps.tile([128, 512], F32, name="bank")
                for j in range(4):
                    nc.tensor.matmul(bank[:], w1q[j][:],
                                     xg[:, 512 * j:512 * (j + 1)],
                                     start=(j == 0), stop=(j == 3))
                if pend:
                    z_tail(*pend.pop())
                T = wp.tile([128, 512], BF16, name="T", bufs=3)
                nc.scalar.activation(T[:], bank[:], AF.Tanh, bias=b1t[:])
                pend.append((g, T))
            z_tail(*pend.pop())

            # ---- beta -> s2d slice ----
            Dd = dsc.tile([8, NG], F32, name="Dd")
            Zd = dsc.tile([8, NG], F32, name="Zd")
            nc.sync.dma_start(Dd[:], Dacc[:])
            nc.sync.dma_start(Zd[:], Zlast[:])
            Dr = gsb.tile([UB, 2], F32, name="Dr")
            Zr = gsb.tile([UB, 2], F32, name="Zr")
            dv = Dd[:].rearrange("(j s) g -> s g j", j=4, s=2)
            zv = Zd[:].rearrange("(j s) g -> s g j", j=4, s=2)
            for s in range(2):
                nc.sync.dma_start(Dr[:, s:s + 1], dv[s])
                nc.sync.dma_start(Zr[:, s:s + 1], zv[s])
            den = gsb.tile([UB, 1], F32, name="den")
            nc.vector.tensor_add(den[:], Dr[:, 0:1], Dr[:, 1:2])
            rec = gsb.tile([UB, 1], F32, name="rec")
            nc.vector.reciprocal(rec[:], den[:])
            betaI = gsb.tile([UB, 1], F32, name="betaI")
            nc.vector.tensor_mul(betaI[:], Zr[:, 1:2], rec[:])
            sterm = gsb.tile([UB, D], F32, name="sterm")
            nc.vector.tensor_scalar(sterm[:], sel_t[:], betaI[:], float(I),
                                    OP.mult, OP.mult)
            s2d_t = gsb.tile([UB, D], F32, name="s2d_t")
            nc.vector.tensor_add(s2d_t[:], sterm[:], ue_t[:])
            nc.sync.dma_start(s2d_out.ap(), s2d_t[:])

            # ---- item GCN (transposed layout, no transposes) + preVT ----
            def gcn_layer(hT_r, fin, fout, w_r, b_t, tag):
                # M chunks natural: lhsT = hT column-slices (small-N MMs)
                mc = []
                for k in range(8):
                    mp = tps.tile([128, fout], F32, name=f"mp_{tag}",
                                  tag="tps")
                    nc.tensor.matmul(mp[:],
                                     hT_r[:, 128 * k:128 * (k + 1)],
                                     w_r[:], start=True, stop=True)
                    c = gsb.tile([128, fout], gdt, name=f"mc_{tag}{k}")
                    nc.vector.tensor_copy(c[:], mp[:])
                    mc.append(c)
                # out = relu(M @ Av.T + b) (fout, I)
                out_r = gsb.tile([fout, I], F32R, name=f"el_{tag}")
                for h in range(2):
                    ops_ = gps.tile([fout, 512], F32, name=f"o_{tag}{h}",
                                    tag="gps2")
                    for k in range(8):
                        nc.tensor.matmul(ops_[:],
                                         mc[k][:],
                                         av_r[k][:, 512 * h:512 * (h + 1)],
                                         start=(k == 0), stop=(k == 7))
                    nc.scalar.activation(out_r[:, 512 * h:512 * (h + 1)],
                                         ops_[:], AF.Relu, bias=b_t[:])
                return out_r

            el1_r = gcn_layer(peT_r, 64, 64, wi1_r, bi1_t, "l1")
            el2_r = gcn_layer(el1_r, 64, 32, wi2_r, bi2_t, "l2")
            el3_r = gcn_layer(el2_r, 32, 16, wi3_r, bi3_t, "l3")

            # elT output: [el1, pe, el2, el1, el3, el2]
            eo = elT_out.ap().bitcast(F32R)
            nc.sync.dma_start(eo[0:64, :], el1_r[:])
            nc.sync.dma_start(elT_out.ap()[64:128, :], peT.ap())
            nc.sync.dma_start(eo[128:160, :], el2_r[:])
            nc.sync.dma_start(eo[160:224, :], el1_r[:])
            nc.sync.dma_start(eo[224:240, :], el3_r[:])
            nc.sync.dma_start(eo[240:272, :], el2_r[:])

            # preVT = lin1b.T @ elT + lin1_b
            parts = [el1_r, peT_r, el2_r, el1_r, el3_r, el2_r]
            pv_sb = gsb.tile([128, I], F32, name="pv_sb")
            for h in range(2):
                pvps = gps.tile([128, 512], F32, name=f"pvps{h}", tag="gps")
                for i, p in enumerate(parts):
                    nc.tensor.matmul(pvps[:],
                                     l1b_r[i][:],
                                     p[:, 512 * h:512 * (h + 1)],
                                     start=(i == 0), stop=(i == 5))
                nc.scalar.activation(pv_sb[:, 512 * h:512 * (h + 1)],
                                     pvps[:], AF.Identity,
                                     bias=l1bias_t[:])
            nc.sync.dma_start(preVT_out.ap(), pv_sb[:])

    nc.compile()
    return nc


# ---------------------------------------------------------------- phase B ---

def build_phase_b(out_b_val):
    nc = bacc.Bacc("TRN2", target_bir_lowering=False, debug=False,
                   num_devices=NCORES)
    dt = nc.dram_tensor
    gdt = BF16 if MODE["gcn_bf16"] else F32R
    sdt = BF16 if MODE["scorer_bf16"] else F32R
    s2dT = dt("s2dT", [D, U], F32, kind="ExternalInput")
    auT = dt("auT", [U, U], gdt, kind="ExternalInput")
    wu1 = dt("wu1", [64, 64], F32, kind="ExternalInput")
    wu2 = dt("wu2", [64, 32], F32, kind="ExternalInput")
    wu3 = dt("wu3", [32, 16], F32, kind="ExternalInput")
    bu1 = dt("bu1", [64, 1], F32, kind="ExternalInput")
    bu2 = dt("bu2", [32, 1], F32, kind="ExternalInput")
    bu3 = dt("bu3", [16, 1], F32, kind="ExternalInput")
    l1a = [dt(f"l1a{i}", [k, 128], F32, kind="ExternalInput")
           for i, k in enumerate([64, 64, 32, 64, 16, 32])]
    preVT = dt("preVT", [128, I], sdt, kind="ExternalInput")
    lin2ab = dt("lin2ab", [2, 128, 128], sdt, kind="ExternalInput")
    lin2b2 = dt("lin2b2", [128, 1], F32, kind="ExternalInput")
    ow4 = dt("ow4", [4, 128, 128], sdt, kind="ExternalInput")

    score_out = dt("score_out", [UB, I], F32, kind="ExternalOutput")
    euT_out = dt("euT_out", [272, U], F32, kind="ExternalOutput")

    with tile.TileContext(nc) as tc:
        with tc.tile_pool(name="cpool", bufs=1) as cp, \
             tc.tile_pool(name="gsb", bufs=1) as gsb, \
             tc.tile_pool(name="spool", bufs=3) as sp, \
             tc.tile_pool(name="scrp", bufs=2) as scp:

            s2dT_r = cp.tile([D, U], F32R, name="s2dT_r")
            nc.sync.dma_start(s2dT_r[:], s2dT.ap().bitcast(F32R))
            au_r = []
            for k in range(4):
                t = cp.tile([128, U], gdt, name=f"au_r{k}")
                nc.sync.dma_start(t[:], auT.ap()[128 * k:128 * (k + 1), :])
                au_r.append(t)
            wu1_r = cp.tile([64, 64], F32R, name="wu1_r")
            nc.sync.dma_start(wu1_r[:], wu1.ap().bitcast(F32R))
            wu2_r = cp.tile([64, 32], F32R, name="wu2_r")
            nc.sync.dma_start(wu2_r[:], wu2.ap().bitcast(F32R))
            wu3_r = cp.tile([32, 16], F32R, name="wu3_r")
            nc.sync.dma_start(wu3_r[:], wu3.ap().bitcast(F32R))
            bu1_t = cp.tile([64, 1], F32, name="bu1_t")
            nc.sync.dma_start(bu1_t[:], bu1.ap())
            bu2_t = cp.tile([32, 1], F32, name="bu2_t")
            nc.sync.dma_start(bu2_t[:], bu2.ap())
            bu3_t = cp.tile([16, 1], F32, name="bu3_t")
            nc.sync.dma_start(bu3_t[:], bu3.ap())
            l1a_r = []
            for i, k in enumerate([64, 64, 32, 64, 16, 32]):
                t = cp.tile([k, 128], F32R, name=f"l1a_r{i}")
                nc.sync.dma_start(t[:], l1a[i].ap().bitcast(F32R))
                l1a_r.append(t)
            pv_sb = cp.tile([128, I], sdt, name="pv_sb")
            nc.sync.dma_start(pv_sb[:], preVT.ap())
            l2ab_r = []
            for a in range(2):
                t = cp.tile([128, 128], sdt, name=f"l2ab_r{a}")
                nc.sync.dma_start(t[:], lin2ab.ap()[a])
                l2ab_r.append(t)
            l2b_t = cp.tile([128, 1], F32, name="l2b_t")
            nc.sync.dma_start(l2b_t[:], lin2b2.ap())
            ow_r = []
            for q in range(4):
                t = cp.tile([128, 128], sdt, name=f"ow_r{q}")
                nc.sync.dma_start(t[:], ow4.ap()[q])
                ow_r.append(t)

            with tc.tile_pool(name="gpsum", bufs=1, space="PSUM") as gps, \
                 tc.tile_pool(name="tpsum", bufs=2, space="PSUM") as tps:
                # ---- user GCN (no transposes) ----
                def gcn_layer(hT_r, fin, fout, w_r, b_t, tag):
                    mc = []
                    for k in range(4):
                        mp = tps.tile([128, fout], F32, name=f"mp_{tag}",
                                      tag="tps")
                        nc.tensor.matmul(mp[:],
                                         hT_r[:, 128 * k:128 * (k + 1)],
                                         w_r[:], start=True, stop=True)
                        c = gsb.tile([128, fout], gdt, name=f"mc_{tag}{k}")
                        nc.vector.tensor_copy(c[:], mp[:])
                        mc.append(c)
                    ops_ = gps.tile([fout, U], F32, name=f"o_{tag}",
                                    tag="gps2")
                    for k in range(4):
                        nc.tensor.matmul(ops_[:], mc[k][:], au_r[k][:],
                                         start=(k == 0), stop=(k == 3))
                    out_r = gsb.tile([fout, U], F32R, name=f"eu_{tag}")
                    nc.scalar.activation(out_r[:], ops_[:], AF.Relu,
                                         bias=b_t[:])
                    return out_r

                eu1_r = gcn_layer(s2dT_r, 64, 64, wu1_r, bu1_t, "l1")
                eu2_r = gcn_layer(eu1_r, 64, 32, wu2_r, bu2_t, "l2")
                eu3_r = gcn_layer(eu2_r, 32, 16, wu3_r, bu3_t, "l3")

                eo = euT_out.ap().bitcast(F32R)
                nc.sync.dma_start(eo[0:64, :], eu1_r[:])
                nc.sync.dma_start(euT_out.ap()[64:128, :], s2dT.ap())
                nc.sync.dma_start(eo[128:160, :], eu2_r[:])
                nc.sync.dma_start(eo[160:224, :], eu1_r[:])
                nc.sync.dma_start(eo[224:240, :], eu3_r[:])
                nc.sync.dma_start(eo[240:272, :], eu2_r[:])

                # preUT (128, 512), no bias (lin1_b folded into preVT)
                parts = [eu1_r, s2dT_r, eu2_r, eu1_r, eu3_r, eu2_r]
                pups = gps.tile([128, U], F32, name="pups", tag="gps")
                for i, p in enumerate(parts):
                    nc.tensor.matmul(pups[:], l1a_r[i][:], p[:],
                                     start=(i == 0), stop=(i == 5))
                pu_sb = gsb.tile([128, U], F32, name="pu_sb")
                nc.vector.tensor_copy(pu_sb[:], pups[:])

            # ---- scorer: 32 pairs of users ----
            # per pair: h1A (DVE) + h1B (ACT); h2 halves accumulate both
            # users into one (128, 512) bank via zero-padded lhsT (all
            # f32r, base partition 0); relu-copy 1 DVE + 1 ACT; combine
            # MMs (deferred one pair) accumulate 4 (pair, half) results
            # into one bank via zero-padded ow lhsT.
            with tc.tile_pool(name="hpsum", bufs=4, space="PSUM") as hps, \
                 tc.tile_pool(name="spsum", bufs=2, space="PSUM") as sps:
                scb = None
                pend = []

                def combine(p, rl):
                    nonlocal scb
                    q = p % 2
                    if q == 0:
                        scb = sps.tile([128, 512], F32, name="scb")
                    for h in range(2):
                        nc.tensor.matmul(scb[:], ow_r[2 * q + h][:],
                                         rl[:, 512 * h:512 * (h + 1)],
                                         start=(q == 0 and h == 0),
                                         stop=(q == 1 and h == 1))
                    if q == 1:
                        scr = scp.tile([128, 512], F32, name="scr")
                        nc.vector.tensor_scalar(scr[:], scb[:],
                                                float(out_b_val), 0.0,
                                                OP.add, OP.max)
                        for q2 in range(2):
                            pp = p - 1 + q2
                            for h in range(2):
                                base = 32 * (2 * q2 + h)
                                nc.gpsimd.dma_start(
                                    score_out.ap()[2 * pp:2 * pp + 2,
                                                   512 * h:512 * (h + 1)],
                                    scr[base:base + 2, :])

                for p in range(UB // 2):
                    uA, uB_ = 2 * p, 2 * p + 1
                    h1A = sp.tile([128, I], sdt, name="h1A")
                    h1B = sp.tile([128, I], sdt, name="h1B")
                    nc.vector.tensor_scalar(h1A[:], pv_sb[:],
                                            pu_sb[:, uA:uA + 1], 0.0,
                                            OP.add, OP.max)
                    if MODE["scorer_bf16"]:
                        # bf16 input -> DVE 4x mode; DVE takes both h1
                        nc.vector.tensor_scalar(h1B[:], pv_sb[:],
                                                pu_sb[:, uB_:uB_ + 1], 0.0,
                                                OP.add, OP.max)
                    else:
                        nc.scalar.activation(h1B[:], pv_sb[:], AF.Relu,
                                             bias=pu_sb[:, uB_:uB_ + 1])
                    rl = sp.tile([128, I], sdt, name="rl")
                    hq0 = hps.tile([128, 512], F32, name="hq0", tag="hq")
                    hq1 = hps.tile([128, 512], F32, name="hq1", tag="hq")
                    nc.tensor.matmul(hq0[:], l2ab_r[0][:], h1A[:, 0:512],
                                     start=True, stop=False)
                    nc.tensor.matmul(hq1[:], l2ab_r[0][:], h1A[:, 512:I],
                                     start=True, stop=False)
                    nc.tensor.matmul(hq0[:], l2ab_r[1][:], h1B[:, 0:512],
                                     start=False, stop=True)
                    nc.tensor.matmul(hq1[:], l2ab_r[1][:], h1B[:, 512:I],
                                     start=False, stop=True)
                    if MODE["scorer_bf16"]:
                        nc.scalar.activation(rl[:, 0:512], hq0[:], AF.Relu,
                                             bias=l2b_t[:])
                    else:
                        nc.vector.tensor_scalar(rl[:, 0:512], hq0[:],
                                                l2b_t[:], 0.0,
                                                OP.add, OP.max)
                    nc.scalar.activation(rl[:, 512:I], hq1[:], AF.Relu,
                                         bias=l2b_t[:])
                    if pend:
                        combine(*pend.pop())
                    pend.append((p, rl))
                combine(*pend.pop())

    nc.compile()
    return nc


# ------------------------------------------------------------- host side ---

_CACHE = {}


def _adj(edges, n):
    e0 = np.asarray(edges[0], np.int64)
    e1 = np.asarray(edges[1], np.int64)
    loops = np.arange(n, dtype=np.int64)
    src = np.concatenate([e0, loops])
    dst = np.concatenate([e1, loops])
    deg = np.bincount(dst, minlength=n).astype(np.float32)
    dinv = np.where(deg > 0, deg ** -0.5, 0.0).astype(np.float32)
    A = np.zeros((n, n), np.float32)
    np.add.at(A, (dst, src), dinv[src] * dinv[dst])
    return A


def prep_phase_a(inputs):
    idx = np.asarray(inputs["u_v_idx"]).astype(np.int64).ravel()
    ui = _f32(inputs["ui_table"])
    user_idx = np.asarray(inputs["user_idx"]).astype(np.int64).ravel()
    item_idx = np.asarray(inputs["item_idx"]).astype(np.int64).ravel()
    ue_full = _f32(inputs["user_table"])[user_idx]
    pe_full = _f32(inputs["item_table"])[item_idx]
    att_W1 = _f32(inputs["att_W1"])
    att_b1 = _f32(inputs["att_b1"])
    att_W2 = _f32(inputs["att_W2"])
    lin1_W = _f32(inputs["lin1_W"])
    lin1_b = _f32(inputs["lin1_b"])

    Av = _adj(np.asarray(inputs["ug_v_edges"]), I)

    # weight blocks
    w1base = np.zeros((128, 32), np.float32)
    w1base[0:64, 0:8] = att_W1
    w1base[64:128, 8:16] = att_W1
    w1blk = np.zeros((4, 128, 128), np.float32)
    for j in range(4):
        w1blk[j][:, 32 * j:32 * (j + 1)] = w1base
    w2sel = np.zeros((128, 8), np.float32)
    for m in range(8):
        j, s = m // 2, m % 2
        w2sel[32 * j + 8 * s:32 * j + 8 * s + 8, m] = att_W2[:, 0]
    b1blk = np.zeros((128, 1), np.float32)
    for j in range(4):
        for s in range(2):
            b1blk[32 * j + 8 * s:32 * j + 8 * s + 8, 0] = att_b1
    part_sizes = [64, 64, 32, 64, 16, 32]
    offs = np.cumsum([0] + part_sizes)
    l1b_parts = [np.ascontiguousarray(lin1_W[272:544][offs[i]:offs[i + 1]])
                 for i in range(6)]

    # phase A inputs
    in_maps_a = []
    seq = idx.reshape(U, I)
    se_last_full = ui[seq[:, -1]]
    for c in range(NCORES):
        Xc = ui[idx[c * POS:(c + 1) * POS]]
        A2 = np.ascontiguousarray(
            Xc.reshape(NT, 2, 512, D).transpose(1, 3, 0, 2)
            .reshape(128, POS // 2))
        in_maps_a.append({
            "xT": _bf16(A2),
            "w1blk": _bf16(w1blk),
            "w2sel": _bf16(w2sel),
            "b1blk": b1blk,
            "se_last": np.ascontiguousarray(
                se_last_full[c * UB:(c + 1) * UB]),
            "ue": np.ascontiguousarray(ue_full[c * UB:(c + 1) * UB]),
            "peT": np.ascontiguousarray(pe_full.T),
            "avT": (_bf16(Av.T) if MODE["gcn_bf16"]
                    else np.ascontiguousarray(Av.T)),
            "wi1": _f32(inputs["igcn_W1"]), "wi2": _f32(inputs["igcn_W2"]),
            "wi3": _f32(inputs["igcn_W3"]),
            "bi1": _f32(inputs["igcn_b1"]).reshape(64, 1),
            "bi2": _f32(inputs["igcn_b2"]).reshape(32, 1),
            "bi3": _f32(inputs["igcn_b3"]).reshape(16, 1),
            **{f"l1b{i}": l1b_parts[i] for i in range(6)},
            "l1bias": lin1_b.reshape(128, 1),
        })
    return in_maps_a


def _sc(x):
    """Convert scorer-path array to the scorer dtype."""
    return _bf16(x) if MODE["scorer_bf16"] else _f32(x)


def prep_phase_b(inputs, s2d_full):
    lin1_W = _f32(inputs["lin1_W"])
    lin2_W = _f32(inputs["lin2_W"])
    lin2_b = _f32(inputs["lin2_b"])
    out_W = _f32(inputs["out_W"])
    Au = _adj(np.asarray(inputs["ug_u_edges"]), U)
    part_sizes = [64, 64, 32, 64, 16, 32]
    offs = np.cumsum([0] + part_sizes)
    l1a_parts = [np.ascontiguousarray(lin1_W[0:272][offs[i]:offs[i + 1]])
                 for i in range(6)]

    # per-core user permutation puts own block first
    lin2ab = np.zeros((2, 128, 128), np.float32)
    lin2ab[0][:, 0:64] = lin2_W
    lin2ab[1][:, 64:128] = lin2_W
    ow4 = np.zeros((4, 128, 128), np.float32)
    for j in range(4):
        ow4[j][0:64, 32 * j] = out_W[:, 0]
        ow4[j][64:128, 32 * j + 1] = out_W[:, 0]
    lin2b2 = np.concatenate([lin2_b, lin2_b]).reshape(128, 1)
    in_maps_b = []
    for c in range(NCORES):
        perm = np.concatenate([
            np.arange(c * UB, (c + 1) * UB),
            np.arange(0, c * UB),
            np.arange((c + 1) * UB, U)]).astype(np.int64)
        Au_p = Au[perm][:, perm]
        s2dT_p = np.ascontiguousarray(s2d_full[perm].T)
        in_maps_b.append({
            "s2dT": s2dT_p,
            "auT": (_bf16(Au_p.T) if MODE["gcn_bf16"]
                    else np.ascontiguousarray(Au_p.T)),
            "wu1": _f32(inputs["ugcn_W1"]), "wu2": _f32(inputs["ugcn_W2"]),
            "wu3": _f32(inputs["ugcn_W3"]),
            "bu1": _f32(inputs["ugcn_b1"]).reshape(64, 1),
            "bu2": _f32(inputs["ugcn_b2"]).reshape(32, 1),
            "bu3": _f32(inputs["ugcn_b3"]).reshape(16, 1),
            **{f"l1a{i}": l1a_parts[i] for i in range(6)},
            "lin2ab": _sc(lin2ab),
            "lin2b2": lin2b2,
            "ow4": _sc(ow4),
        })
    return in_maps_b


def kernel(**inputs):
    in_maps_a = prep_phase_a(inputs)
    if "A" not in _CACHE:
        _CACHE["A"] = build_phase_a()
    res_a = run_bass_kernel_spmd(_CACHE["A"], in_maps_a,
                                 core_ids=list(range(NCORES)))

    s2d_full = np.concatenate([res_a.results[c]["s2d_out"]
                               for c in range(NCORES)], axis=0)
    elT = res_a.results[0]["elT_out"]
    preVT = res_a.results[0]["preVT_out"]

    in_maps_b = prep_phase_b(inputs, s2d_full)
    for m in in_maps_b:
        m["preVT"] = _sc(preVT)

    out_b = _f32(inputs["out_b"])
    if "B" not in _CACHE:
        _CACHE["B"] = build_phase_b(float(out_b.ravel()[0]))
    res_b = run_bass_kernel_spmd(_CACHE["B"], in_maps_b,
                                 core_ids=list(range(NCORES)))

    score = np.concatenate([res_b.results[c]["score_out"]
                            for c in range(NCORES)], axis=0)
    euT = res_b.results[0]["euT_out"]
    eu = np.ascontiguousarray(euT.T)
    el = np.ascontiguousarray(elT.T)
    return score, eu, el


# revision 28
# speedup vs baseline: 1.0941x; 1.0941x over previous
"""Trainium2 Bass kernel for nn_FGCF (attention + GCN + all-pairs MLP scorer).

Self-contained: takes FULL inputs, shards across 8 NeuronCores internally,
returns FULL outputs (score, eu, el).

Design (2 NEFF launches, no collectives):
  Phase A (users sharded by core; item path replicated):
    - attention MLP z = tanh(se@W1+b1)@W2 over each core's 65536 gathered
      rows (host-gathered + transposed + bf16, as the user-axis shard),
      softmax-denominator per user via ACT exp+accum -> beta -> s2d slice.
    - item GCN (dense normalized adjacency, transposed layout) -> elT,
      pre_vT = lin1b.T @ elT + b1.
  Host: concat s2d slices, permute user axis per core.
  Phase B (users sharded; GCN replicated):
    - user GCN -> euT, pre_uT
    - all-pairs scorer: h1T = relu(pre_vT + pre_uT[:,u]) (ACT/DVE split),
      h2T = lin2.T @ h1T (PE, f32r, 2 users packed via col-groups),
      score = relu(outW.T @ relu(h2T) + out_b) (PE block-diag + fused bias).
"""
import numpy as np
import ml_dtypes

import concourse.bacc as bacc
import concourse.tile as tile
import concourse.mybir as mybir
from concourse.bass_utils import run_bass_kernel_spmd

F32 = mybir.dt.float32
F32R = mybir.dt.float32r
BF16 = mybir.dt.bfloat16
AF = mybir.ActivationFunctionType
OP = mybir.AluOpType

# precision/speed tiers: scorer_bf16 trades ~3e-3 score rel-err for ~25us;
# gcn_bf16 trades ~1e-3 eu/el rel-err for ~15us.
MODE = {"scorer_bf16": False, "gcn_bf16": False}

U, I, D = 512, 1024, 64
NCORES = 8
UB = U // NCORES          # 64 users per core
POS = UB * I              # 65536 positions per core
NT = UB                   # 64 tiles (1 user each): (128, 512) two-chunk stack
NG = NT // 4              # 16 groups of 4 tiles


def _f32(x):
    return np.ascontiguousarray(np.asarray(x, dtype=np.float32))


def _bf16(x):
    return np.ascontiguousarray(np.asarray(x).astype(ml_dtypes.bfloat16))


# ---------------------------------------------------------------- phase A ---

def build_phase_a():
    nc = bacc.Bacc("TRN2", target_bir_lowering=False, debug=False,
                   num_devices=NCORES)
    dt = nc.dram_tensor
    xT = dt("xT", [128, POS // 2], BF16, kind="ExternalInput")
    w1blk = dt("w1blk", [128, 32], BF16, kind="ExternalInput")
    w2sel = dt("w2sel", [128, 8], BF16, kind="ExternalInput")
    b1blk = dt("b1blk", [128, 1], F32, kind="ExternalInput")
    se_last = dt("se_last", [UB, D], F32, kind="ExternalInput")
    ue = dt("ue", [UB, D], F32, kind="ExternalInput")
    peT = dt("peT", [D, I], F32, kind="ExternalInput")
    gdt = BF16 if MODE["gcn_bf16"] else F32R
    avT = dt("avT", [I, I], gdt, kind="ExternalInput")
    wi1 = dt("wi1", [64, 64], F32, kind="ExternalInput")
    wi2 = dt("wi2", [64, 32], F32, kind="ExternalInput")
    wi3 = dt("wi3", [32, 16], F32, kind="ExternalInput")
    bi1 = dt("bi1", [64, 1], F32, kind="ExternalInput")
    bi2 = dt("bi2", [32, 1], F32, kind="ExternalInput")
    bi3 = dt("bi3", [16, 1], F32, kind="ExternalInput")
    l1b = [dt(f"l1b{i}", [k, 128], F32, kind="ExternalInput")
           for i, k in enumerate([64, 64, 32, 64, 16, 32])]
    l1bias = dt("l1bias", [128, 1], F32, kind="ExternalInput")

    s2d_out = dt("s2d_out", [UB, D], F32, kind="ExternalOutput")
    elT_out = dt("elT_out", [272, I], F32, kind="ExternalOutput")
    preVT_out = dt("preVT_out", [128, I], F32, kind="ExternalOutput")

    with tile.TileContext(nc) as tc:
        with tc.tile_pool(name="cpool", bufs=1) as cp, \
             tc.tile_pool(name="xpool", bufs=3) as xp, \
             tc.tile_pool(name="wpool", bufs=2) as wp, \
             tc.tile_pool(name="gsb", bufs=1) as gsb, \
             tc.tile_pool(name="apsum", bufs=2, space="PSUM") as aps, \
             tc.tile_pool(name="zpsum", bufs=2, space="PSUM") as zps, \
             tc.tile_pool(name="gpsum", bufs=1, space="PSUM") as gps, \
             tc.tile_pool(name="tpsum", bufs=2, space="PSUM") as tps, \
             tc.tile_pool(name="dscr", bufs=1, space="DRAM") as dsc:

            # ---- constants ----
            w1 = cp.tile([128, 32], BF16, name="w1")
            nc.sync.dma_start(w1[:], w1blk.ap())
            w2 = cp.tile([128, 8], BF16, name="w2")
            nc.sync.dma_start(w2[:], w2sel.ap())
            b1t = cp.tile([128, 1], F32, name="b1t")
            nc.sync.dma_start(b1t[:], b1blk.ap())
            sel_t = cp.tile([UB, D], F32, name="sel_t")
            nc.sync.dma_start(sel_t[:], se_last.ap())
            ue_t = cp.tile([UB, D], F32, name="ue_t")
            nc.sync.dma_start(ue_t[:], ue.ap())
            peT_r = cp.tile([D, I], F32R, name="peT_r")
            nc.sync.dma_start(peT_r[:], peT.ap().bitcast(F32R))
            wi1_r = cp.tile([64, 64], F32R, name="wi1_r")
            nc.sync.dma_start(wi1_r[:], wi1.ap().bitcast(F32R))
            wi2_r = cp.tile([64, 32], F32R, name="wi2_r")
            nc.sync.dma_start(wi2_r[:], wi2.ap().bitcast(F32R))
            wi3_r = cp.tile([32, 16], F32R, name="wi3_r")
            nc.sync.dma_start(wi3_r[:], wi3.ap().bitcast(F32R))
            bi1_t = cp.tile([64, 1], F32, name="bi1_t")
            nc.sync.dma_start(bi1_t[:], bi1.ap())
            bi2_t = cp.tile([32, 1], F32, name="bi2_t")
            nc.sync.dma_start(bi2_t[:], bi2.ap())
            bi3_t = cp.tile([16, 1], F32, name="bi3_t")
            nc.sync.dma_start(bi3_t[:], bi3.ap())
            l1b_r = []
            for i, k in enumerate([64, 64, 32, 64, 16, 32]):
                t = cp.tile([k, 128], F32R, name=f"l1b_r{i}")
                nc.sync.dma_start(t[:], l1b[i].ap().bitcast(F32R))
                l1b_r.append(t)
            l1bias_t = cp.tile([128, 1], F32, name="l1bias_t")
            nc.sync.dma_start(l1bias_t[:], l1bias.ap())
            av_r = []
            for k in range(8):
                t = cp.tile([128, I], gdt, name=f"av_r{k}")
                nc.sync.dma_start(t[:], avT.ap()[128 * k:128 * (k + 1), :])
                av_r.append(t)

            # ---- attention: z + softmax denominators ----
            # software-pipelined: W2-combine/exp for group g-1 are emitted
            # between group g's matmuls so the PE stream never stalls.
            Dacc = gsb.tile([8, NG], F32, name="Dacc")
            Zlast = gsb.tile([8, NG], F32, name="Zlast")
            pend = []

            def z_tail(g, T):
                zb = zps.tile([8, 512], F32, name="zb")
                nc.tensor.matmul(zb[:], w2[:], T[:], start=True, stop=True)
                E = wp.tile([8, 512], F32, name="E")
                nc.scalar.activation(E[:], zb[:], AF.Exp,
                                     accum_out=Dacc[:, g:g + 1])
                nc.vector.tensor_copy(Zlast[:, g:g + 1], E[:, 511:512])

            for g in range(NG):
                xg = xp.tile([128, 2048], BF16, name="xg")
                nc.sync.dma_start(xg[:], xT.ap()[:, 2048 * g:2048 * (g + 1)])
                bank = aps.tile([128, 512], F32, name="bank")
                for j in range(4):
                    nc.tensor.matmul(bank[32 * j:32 * j + 32, :], w1[:],
                                     xg[:, 512 * j:512 * (j + 1)],
                                     start=True, stop=True,
                                     tile_position=(0, 32 * j))
                if pend:
                    z_tail(*pend.pop())
                T = wp.tile([128, 512], BF16, name="T", bufs=3)
                nc.scalar.activation(T[:], bank[:], AF.Tanh, bias=b1t[:])
                pend.append((g, T))
            z_tail(*pend.pop())

            # ---- beta -> s2d slice ----
            Dd = dsc.tile([8, NG], F32, name="Dd")
            Zd = dsc.tile([8, NG], F32, name="Zd")
            nc.sync.dma_start(Dd[:], Dacc[:])
            nc.sync.dma_start(Zd[:], Zlast[:])
            Dr = gsb.tile([UB, 2], F32, name="Dr")
            Zr = gsb.tile([UB, 2], F32, name="Zr")
            dv = Dd[:].rearrange("(j s) g -> s g j", j=4, s=2)
            zv = Zd[:].rearrange("(j s) g -> s g j", j=4, s=2)
            for s in range(2):
                nc.sync.dma_start(Dr[:, s:s + 1], dv[s])
                nc.sync.dma_start(Zr[:, s:s + 1], zv[s])
            den = gsb.tile([UB, 1], F32, name="den")
            nc.vector.tensor_add(den[:], Dr[:, 0:1], Dr[:, 1:2])
            rec = gsb.tile([UB, 1], F32, name="rec")
            nc.vector.reciprocal(rec[:], den[:])
            betaI = gsb.tile([UB, 1], F32, name="betaI")
            nc.vector.tensor_mul(betaI[:], Zr[:, 1:2], rec[:])
            sterm = gsb.tile([UB, D], F32, name="sterm")
            nc.vector.tensor_scalar(sterm[:], sel_t[:], betaI[:], float(I),
                                    OP.mult, OP.mult)
            s2d_t = gsb.tile([UB, D], F32, name="s2d_t")
            nc.vector.tensor_add(s2d_t[:], sterm[:], ue_t[:])
            nc.sync.dma_start(s2d_out.ap(), s2d_t[:])

            # ---- item GCN (transposed layout, no transposes) + preVT ----
            def gcn_layer(hT_r, fin, fout, w_r, b_t, tag):
                # M chunks natural: lhsT = hT column-slices (small-N MMs)
                mc = []
                for k in range(8):
                    mp = tps.tile([128, fout], F32, name=f"mp_{tag}",
                                  tag="tps")
                    nc.tensor.matmul(mp[:],
                                     hT_r[:, 128 * k:128 * (k + 1)],
                                     w_r[:], start=True, stop=True)
                    c = gsb.tile([128, fout], gdt, name=f"mc_{tag}{k}")
                    nc.vector.tensor_copy(c[:], mp[:])
                    mc.append(c)
                # out = relu(M @ Av.T + b) (fout, I)
                out_r = gsb.tile([fout, I], F32R, name=f"el_{tag}")
                for h in range(2):
                    ops_ = gps.tile([fout, 512], F32, name=f"o_{tag}{h}",
                                    tag="gps2")
                    for k in range(8):
                        nc.tensor.matmul(ops_[:],
                                         mc[k][:],
                                         av_r[k][:, 512 * h:512 * (h + 1)],
                                         start=(k == 0), stop=(k == 7))
                    nc.scalar.activation(out_r[:, 512 * h:512 * (h + 1)],
                                         ops_[:], AF.Relu, bias=b_t[:])
                return out_r

            el1_r = gcn_layer(peT_r, 64, 64, wi1_r, bi1_t, "l1")
            el2_r = gcn_layer(el1_r, 64, 32, wi2_r, bi2_t, "l2")
            el3_r = gcn_layer(el2_r, 32, 16, wi3_r, bi3_t, "l3")

            # elT output: [el1, pe, el2, el1, el3, el2]
            eo = elT_out.ap().bitcast(F32R)
            nc.sync.dma_start(eo[0:64, :], el1_r[:])
            nc.sync.dma_start(elT_out.ap()[64:128, :], peT.ap())
            nc.sync.dma_start(eo[128:160, :], el2_r[:])
            nc.sync.dma_start(eo[160:224, :], el1_r[:])
            nc.sync.dma_start(eo[224:240, :], el3_r[:])
            nc.sync.dma_start(eo[240:272, :], el2_r[:])

            # preVT = lin1b.T @ elT + lin1_b
            parts = [el1_r, peT_r, el2_r, el1_r, el3_r, el2_r]
            pv_sb = gsb.tile([128, I], F32, name="pv_sb")
            for h in range(2):
                pvps = gps.tile([128, 512], F32, name=f"pvps{h}", tag="gps")
                for i, p in enumerate(parts):
                    nc.tensor.matmul(pvps[:],
                                     l1b_r[i][:],
                                     p[:, 512 * h:512 * (h + 1)],
                                     start=(i == 0), stop=(i == 5))
                nc.scalar.activation(pv_sb[:, 512 * h:512 * (h + 1)],
                                     pvps[:], AF.Identity,
                                     bias=l1bias_t[:])
            nc.sync.dma_start(preVT_out.ap(), pv_sb[:])

    nc.compile()
    return nc


# ---------------------------------------------------------------- phase B ---

def build_phase_b(out_b_val):
    nc = bacc.Bacc("TRN2", target_bir_lowering=False, debug=False,
                   num_devices=NCORES)
    dt = nc.dram_tensor
    gdt = BF16 if MODE["gcn_bf16"] else F32R
    sdt = BF16 if MODE["scorer_bf16"] else F32R
    s2dT = dt("s2dT", [D, U], F32, kind="ExternalInput")
    auT = dt("auT", [U, U], gdt, kind="ExternalInput")
    wu1 = dt("wu1", [64, 64], F32, kind="ExternalInput")
    wu2 = dt("wu2", [64, 32], F32, kind="ExternalInput")
    wu3 = dt("wu3", [32, 16], F32, kind="ExternalInput")
    bu1 = dt("bu1", [64, 1], F32, kind="ExternalInput")
    bu2 = dt("bu2", [32, 1], F32, kind="ExternalInput")
    bu3 = dt("bu3", [16, 1], F32, kind="ExternalInput")
    l1a = [dt(f"l1a{i}", [k, 128], F32, kind="ExternalInput")
           for i, k in enumerate([64, 64, 32, 64, 16, 32])]
    preVT = dt("preVT", [128, I], sdt, kind="ExternalInput")
    lin2ab = dt("lin2ab", [2, 128, 128], sdt, kind="ExternalInput")
    lin2b2 = dt("lin2b2", [128, 1], F32, kind="ExternalInput")
    ow4 = dt("ow4", [4, 128, 128], sdt, kind="ExternalInput")

    score_out = dt("score_out", [UB, I], F32, kind="ExternalOutput")
    euT_out = dt("euT_out", [272, U], F32, kind="ExternalOutput")

    with tile.TileContext(nc) as tc:
        with tc.tile_pool(name="cpool", bufs=1) as cp, \
             tc.tile_pool(name="gsb", bufs=1) as gsb, \
             tc.tile_pool(name="spool", bufs=3) as sp, \
             tc.tile_pool(name="scrp", bufs=2) as scp:

            s2dT_r = cp.tile([D, U], F32R, name="s2dT_r")
            nc.sync.dma_start(s2dT_r[:], s2dT.ap().bitcast(F32R))
            au_r = []
            for k in range(4):
                t = cp.tile([128, U], gdt, name=f"au_r{k}")
                nc.sync.dma_start(t[:], auT.ap()[128 * k:128 * (k + 1), :])
                au_r.append(t)
            wu1_r = cp.tile([64, 64], F32R, name="wu1_r")
            nc.sync.dma_start(wu1_r[:], wu1.ap().bitcast(F32R))
            wu2_r = cp.tile([64, 32], F32R, name="wu2_r")
            nc.sync.dma_start(wu2_r[:], wu2.ap().bitcast(F32R))
            wu3_r = cp.tile([32, 16], F32R, name="wu3_r")
            nc.sync.dma_start(wu3_r[:], wu3.ap().bitcast(F32R))
            bu1_t = cp.tile([64, 1], F32, name="bu1_t")
            nc.sync.dma_start(bu1_t[:], bu1.ap())
            bu2_t = cp.tile([32, 1], F32, name="bu2_t")
            nc.sync.dma_start(bu2_t[:], bu2.ap())
            bu3_t = cp.tile([16, 1], F32, name="bu3_t")
            nc.sync.dma_start(bu3_t[:], bu3.ap())
            l1a_r = []
            for i, k in enumerate([64, 64, 32, 64, 16, 32]):
                t = cp.tile([k, 128], F32R, name=f"l1a_r{i}")
                nc.sync.dma_start(t[:], l1a[i].ap().bitcast(F32R))
                l1a_r.append(t)
            pv_sb = cp.tile([128, I], sdt, name="pv_sb")
            nc.sync.dma_start(pv_sb[:], preVT.ap())
            l2ab_r = []
            for a in range(2):
                t = cp.tile([128, 128], sdt, name=f"l2ab_r{a}")
                nc.sync.dma_start(t[:], lin2ab.ap()[a])
                l2ab_r.append(t)
            l2b_t = cp.tile([128, 1], F32, name="l2b_t")
            nc.sync.dma_start(l2b_t[:], lin2b2.ap())
            ow_r = []
            for q in range(4):
                t = cp.tile([128, 128], sdt, name=f"ow_r{q}")
                nc.sync.dma_start(t[:], ow4.ap()[q])
                ow_r.append(t)

            with tc.tile_pool(name="gpsum", bufs=1, space="PSUM") as gps, \
                 tc.tile_pool(name="tpsum", bufs=2, space="PSUM") as tps:
                # ---- user GCN (no transposes) ----
                def gcn_layer(hT_r, fin, fout, w_r, b_t, tag):
                    mc = []
                    for k in range(4):
                        mp = tps.tile([128, fout], F32, name=f"mp_{tag}",
                                      tag="tps")
                        nc.tensor.matmul(mp[:],
                                         hT_r[:, 128 * k:128 * (k + 1)],
                                         w_r[:], start=True, stop=True)
                        c = gsb.tile([128, fout], gdt, name=f"mc_{tag}{k}")
                        nc.vector.tensor_copy(c[:], mp[:])
                        mc.append(c)
                    ops_ = gps.tile([fout, U], F32, name=f"o_{tag}",
                                    tag="gps2")
                    for k in range(4):
                        nc.tensor.matmul(ops_[:], mc[k][:], au_r[k][:],
                                         start=(k == 0), stop=(k == 3))
                    out_r = gsb.tile([fout, U], F32R, name=f"eu_{tag}")
                    nc.scalar.activation(out_r[:], ops_[:], AF.Relu,
                                         bias=b_t[:])
                    return out_r

                eu1_r = gcn_layer(s2dT_r, 64, 64, wu1_r, bu1_t, "l1")
                eu2_r = gcn_layer(eu1_r, 64, 32, wu2_r, bu2_t, "l2")
                eu3_r = gcn_layer(eu2_r, 32, 16, wu3_r, bu3_t, "l3")

                eo = euT_out.ap().bitcast(F32R)
                nc.sync.dma_start(eo[0:64, :], eu1_r[:])
                nc.sync.dma_start(euT_out.ap()[64:128, :], s2dT.ap())
                nc.sync.dma_start(eo[128:160, :], eu2_r[:])
                nc.sync.dma_start(eo[160:224, :], eu1_r[:])
                nc.sync.dma_start(eo[224:240, :], eu3_r[:])
                nc.sync.dma_start(eo[240:272, :], eu2_r[:])

                # preUT (128, 512), no bias (lin1_b folded into preVT)
                parts = [eu1_r, s2dT_r, eu2_r, eu1_r, eu3_r, eu2_r]
                pups = gps.tile([128, U], F32, name="pups", tag="gps")
                for i, p in enumerate(parts):
                    nc.tensor.matmul(pups[:], l1a_r[i][:], p[:],
                                     start=(i == 0), stop=(i == 5))
                pu_sb = gsb.tile([128, U], F32, name="pu_sb")
                nc.vector.tensor_copy(pu_sb[:], pups[:])

            # ---- scorer: 32 pairs of users ----
            # per pair: h1A (DVE) + h1B (ACT); h2 halves accumulate both
            # users into one (128, 512) bank via zero-padded lhsT (all
            # f32r, base partition 0); relu-copy 1 DVE + 1 ACT; combine
            # MMs (deferred one pair) accumulate 4 (pair, half) results
            # into one bank via zero-padded ow lhsT.
            with tc.tile_pool(name="hpsum", bufs=4, space="PSUM") as hps, \
                 tc.tile_pool(name="spsum", bufs=2, space="PSUM") as sps:
                scb = None
                pend = []

                def combine(p, rl):
                    nonlocal scb
                    q = p % 2
                    if q == 0:
                        scb = sps.tile([128, 512], F32, name="scb")
                    for h in range(2):
                        nc.tensor.matmul(scb[:], ow_r[2 * q + h][:],
                                         rl[:, 512 * h:512 * (h + 1)],
                                         start=(q == 0 and h == 0),
                                         stop=(q == 1 and h == 1))
                    if q == 1:
                        scr = scp.tile([128, 512], F32, name="scr")
                        nc.vector.tensor_scalar(scr[:], scb[:],
                                                float(out_b_val), 0.0,
                                                OP.add, OP.max)
                        for q2 in range(2):
                            pp = p - 1 + q2
                            for h in range(2):
                                base = 32 * (2 * q2 + h)
                                nc.gpsimd.dma_start(
                                    score_out.ap()[2 * pp:2 * pp + 2,
                                                   512 * h:512 * (h + 1)],
                                    scr[base:base + 2, :])

                for p in range(UB // 2):
                    uA, uB_ = 2 * p, 2 * p + 1
                    h1A = sp.tile([128, I], sdt, name="h1A")
                    h1B = sp.tile([128, I], sdt, name="h1B")
                    nc.vector.tensor_scalar(h1A[:], pv_sb[:],
                                            pu_sb[:, uA:uA + 1], 0.0,
                                            OP.add, OP.max)
                    if MODE["scorer_bf16"]:
                        # bf16 input -> DVE 4x mode; DVE takes both h1
                        nc.vector.tensor_scalar(h1B[:], pv_sb[:],
                                                pu_sb[:, uB_:uB_ + 1], 0.0,
                                                OP.add, OP.max)
                    else:
                        nc.scalar.activation(h1B[:], pv_sb[:], AF.Relu,
                                             bias=pu_sb[:, uB_:uB_ + 1])
                    rl = sp.tile([128, I], sdt, name="rl")
                    hq0 = hps.tile([128, 512], F32, name="hq0", tag="hq")
                    hq1 = hps.tile([128, 512], F32, name="hq1", tag="hq")
                    nc.tensor.matmul(hq0[:], l2ab_r[0][:], h1A[:, 0:512],
                                     start=True, stop=False)
                    nc.tensor.matmul(hq1[:], l2ab_r[0][:], h1A[:, 512:I],
                                     start=True, stop=False)
                    nc.tensor.matmul(hq0[:], l2ab_r[1][:], h1B[:, 0:512],
                                     start=False, stop=True)
                    nc.tensor.matmul(hq1[:], l2ab_r[1][:], h1B[:, 512:I],
                                     start=False, stop=True)
                    if MODE["scorer_bf16"]:
                        nc.scalar.activation(rl[:, 0:512], hq0[:], AF.Relu,
                                             bias=l2b_t[:])
                    else:
                        nc.vector.tensor_scalar(rl[:, 0:512], hq0[:],
                                                l2b_t[:], 0.0,
                                                OP.add, OP.max)
                    nc.scalar.activation(rl[:, 512:I], hq1[:], AF.Relu,
                                         bias=l2b_t[:])
                    if pend:
                        combine(*pend.pop())
                    pend.append((p, rl))
                combine(*pend.pop())

    nc.compile()
    return nc


# ------------------------------------------------------------- host side ---

_CACHE = {}


def _adj(edges, n):
    e0 = np.asarray(edges[0], np.int64)
    e1 = np.asarray(edges[1], np.int64)
    loops = np.arange(n, dtype=np.int64)
    src = np.concatenate([e0, loops])
    dst = np.concatenate([e1, loops])
    deg = np.bincount(dst, minlength=n).astype(np.float32)
    dinv = np.where(deg > 0, deg ** -0.5, 0.0).astype(np.float32)
    A = np.zeros((n, n), np.float32)
    np.add.at(A, (dst, src), dinv[src] * dinv[dst])
    return A


def prep_phase_a(inputs):
    idx = np.asarray(inputs["u_v_idx"]).astype(np.int64).ravel()
    ui = _f32(inputs["ui_table"])
    user_idx = np.asarray(inputs["user_idx"]).astype(np.int64).ravel()
    item_idx = np.asarray(inputs["item_idx"]).astype(np.int64).ravel()
    ue_full = _f32(inputs["user_table"])[user_idx]
    pe_full = _f32(inputs["item_table"])[item_idx]
    att_W1 = _f32(inputs["att_W1"])
    att_b1 = _f32(inputs["att_b1"])
    att_W2 = _f32(inputs["att_W2"])
    lin1_W = _f32(inputs["lin1_W"])
    lin1_b = _f32(inputs["lin1_b"])

    Av = _adj(np.asarray(inputs["ug_v_edges"]), I)

    # weight blocks
    w1blk = np.zeros((128, 32), np.float32)
    w1blk[0:64, 0:8] = att_W1
    w1blk[64:128, 8:16] = att_W1
    w2sel = np.zeros((128, 8), np.float32)
    for m in range(8):
        j, s = m // 2, m % 2
        w2sel[32 * j + 8 * s:32 * j + 8 * s + 8, m] = att_W2[:, 0]
    b1blk = np.zeros((128, 1), np.float32)
    for j in range(4):
        for s in range(2):
            b1blk[32 * j + 8 * s:32 * j + 8 * s + 8, 0] = att_b1
    part_sizes = [64, 64, 32, 64, 16, 32]
    offs = np.cumsum([0] + part_sizes)
    l1b_parts = [np.ascontiguousarray(lin1_W[272:544][offs[i]:offs[i + 1]])
                 for i in range(6)]

    # phase A inputs
    in_maps_a = []
    seq = idx.reshape(U, I)
    se_last_full = ui[seq[:, -1]]
    for c in range(NCORES):
        Xc = ui[idx[c * POS:(c + 1) * POS]]
        A2 = np.ascontiguousarray(
            Xc.reshape(NT, 2, 512, D).transpose(1, 3, 0, 2)
            .reshape(128, POS // 2))
        in_maps_a.append({
            "xT": _bf16(A2),
            "w1blk": _bf16(w1blk),
            "w2sel": _bf16(w2sel),
            "b1blk": b1blk,
            "se_last": np.ascontiguousarray(
                se_last_full[c * UB:(c + 1) * UB]),
            "ue": np.ascontiguousarray(ue_full[c * UB:(c + 1) * UB]),
            "peT": np.ascontiguousarray(pe_full.T),
            "avT": (_bf16(Av.T) if MODE["gcn_bf16"]
                    else np.ascontiguousarray(Av.T)),
            "wi1": _f32(inputs["igcn_W1"]), "wi2": _f32(inputs["igcn_W2"]),
            "wi3": _f32(inputs["igcn_W3"]),
            "bi1": _f32(inputs["igcn_b1"]).reshape(64, 1),
            "bi2": _f32(inputs["igcn_b2"]).reshape(32, 1),
            "bi3": _f32(inputs["igcn_b3"]).reshape(16, 1),
            **{f"l1b{i}": l1b_parts[i] for i in range(6)},
            "l1bias": lin1_b.reshape(128, 1),
        })
    return in_maps_a


def _sc(x):
    """Convert scorer-path array to the scorer dtype."""
    return _bf16(x) if MODE["scorer_bf16"] else _f32(x)


def prep_phase_b(inputs, s2d_full):
    lin1_W = _f32(inputs["lin1_W"])
    lin2_W = _f32(inputs["lin2_W"])
    lin2_b = _f32(inputs["lin2_b"])
    out_W = _f32(inputs["out_W"])
    Au = _adj(np.asarray(inputs["ug_u_edges"]), U)
    part_sizes = [64, 64, 32, 64, 16, 32]
    offs = np.cumsum([0] + part_sizes)
    l1a_parts = [np.ascontiguousarray(lin1_W[0:272][offs[i]:offs[i + 1]])
                 for i in range(6)]

    # per-core user permutation puts own block first
    lin2ab = np.zeros((2, 128, 128), np.float32)
    lin2ab[0][:, 0:64] = lin2_W
    lin2ab[1][:, 64:128] = lin2_W
    ow4 = np.zeros((4, 128, 128), np.float32)
    for j in range(4):
        ow4[j][0:64, 32 * j] = out_W[:, 0]
        ow4[j][64:128, 32 * j + 1] = out_W[:, 0]
    lin2b2 = np.concatenate([lin2_b, lin2_b]).reshape(128, 1)
    in_maps_b = []
    for c in range(NCORES):
        perm = np.concatenate([
            np.arange(c * UB, (c + 1) * UB),
            np.arange(0, c * UB),
            np.arange((c + 1) * UB, U)]).astype(np.int64)
        Au_p = Au[perm][:, perm]
        s2dT_p = np.ascontiguousarray(s2d_full[perm].T)
        in_maps_b.append({
            "s2dT": s2dT_p,
            "auT": (_bf16(Au_p.T) if MODE["gcn_bf16"]
                    else np.ascontiguousarray(Au_p.T)),
            "wu1": _f32(inputs["ugcn_W1"]), "wu2": _f32(inputs["ugcn_W2"]),
            "wu3": _f32(inputs["ugcn_W3"]),
            "bu1": _f32(inputs["ugcn_b1"]).reshape(64, 1),
            "bu2": _f32(inputs["ugcn_b2"]).reshape(32, 1),
            "bu3": _f32(inputs["ugcn_b3"]).reshape(16, 1),
            **{f"l1a{i}": l1a_parts[i] for i in range(6)},
            "lin2ab": _sc(lin2ab),
            "lin2b2": lin2b2,
            "ow4": _sc(ow4),
        })
    return in_maps_b


def kernel(**inputs):
    in_maps_a = prep_phase_a(inputs)
    if "A" not in _CACHE:
        _CACHE["A"] = build_phase_a()
    res_a = run_bass_kernel_spmd(_CACHE["A"], in_maps_a,
                                 core_ids=list(range(NCORES)))

    s2d_full = np.concatenate([res_a.results[c]["s2d_out"]
                               for c in range(NCORES)], axis=0)
    elT = res_a.results[0]["elT_out"]
    preVT = res_a.results[0]["preVT_out"]

    in_maps_b = prep_phase_b(inputs, s2d_full)
    for m in in_maps_b:
        m["preVT"] = _sc(preVT)

    out_b = _f32(inputs["out_b"])
    if "B" not in _CACHE:
        _CACHE["B"] = build_phase_b(float(out_b.ravel()[0]))
    res_b = run_bass_kernel_spmd(_CACHE["B"], in_maps_b,
                                 core_ids=list(range(NCORES)))

    score = np.concatenate([res_b.results[c]["score_out"]
                            for c in range(NCORES)], axis=0)
    euT = res_b.results[0]["euT_out"]
    eu = np.ascontiguousarray(euT.T)
    el = np.ascontiguousarray(elT.T)
    return score, eu, el
